# revision 7
# baseline (speedup 1.0000x reference)
"""Trainium2 Bass/Tile kernel: batched dot-product attention with length masking.

Problem: queries/keys/values [32, 1024, 128] f32, valid_length [32] int64.
  out = softmax(mask(Q K^T / sqrt(128))) @ V

Strategy:
  - Data-parallel: 32 batches sharded 4-per-core across 8 NeuronCores (SPMD,
    identical program, per-core input maps).
  - Host prep per batch (layout only, so every DMA moves 2-4KB contiguous
    chunks per partition):
      qT/kT = Q^T/K^T    [128=D, 1024] f32 (contraction dim on partitions)
      vsh[p, kb, v] = V[kb*128+p, v]  fp16, partition-major
      mbias[p, b*8+kb] = 0 if kb*128+p < vl[b] else -1e9   (exp bias mask)
  - Device per batch (matmul passes stream 512-row moving operands so the
    PE keeps its stationary loaded across 1024 rows):
      S^T[k, q] = (K^T_kb).T @ Q^T          fp16 matmul, full PE rate
      P^T_kb    = exp(S^T * 1/sqrt(D) + mbias_col)  ScalarE, PSUM->SBUF, fp16.
                  The per-partition bias column zeroes masked k rows exactly
                  (exp(-1e9) == 0), so no V masking or mask matmul is needed.
                  No rowmax: scores ~ N(0,1), |S| <~ 6.
      acc       = sum_kb P^T_kb             running adds, DVE/GpSimd alternate
      den[1,q]  = ones.T @ acc              (PE, [128,1] ones stationary: one
                                             1024-row stream per batch instead
                                             of the old KB*1024-row mask pass)
      O^T[v,q]  = sum_kb V_kb @ P^T_kb      (PE, V stationary)
    O^T (unnormalized) and den are DMAed out; the host does out = O^T.T/den.
  - Warmup: ~12 small [1,256] matmuls (ones stationary, memset source) keep
    the PE busy from ~6.5us so the p-state ramp (3us continuous -> 2.4GHz)
    completes right as batch-0 data lands; they write the den PSUM bank so
    no extra PSUM pressure and no gpsimd memset on the DMA-issue path.
  - Batch-0 latency: k-block-0 goes down as its own 32KB DMA ahead of q, so
    the first real S matmul starts as early as possible.
  - Length specialization: batches sorted by valid_length desc, assigned
    round-robin so slot j is similar across cores; program compiled per
    kb_counts skips fully-masked k-blocks.
"""

import os

import numpy as np
import ml_dtypes

import concourse.tile as tile
from concourse import bacc, mybir
from concourse.bass_utils import run_bass_kernel_spmd

B, Q, K, D = 32, 1024, 1024, 128
N_CORES = 8
BPC = B // N_CORES  # batches per core
KB_MAX = K // 128
QH = 512
SCALE = float(1.0 / np.sqrt(D))
N_WARM = 22

# Matmul operand dtype. fp16: 1 cyc/row PE rate with 10-bit mantissa (S-score
# abs err ~5e-4 — exp/fp16-P error dominates); f32r/f32 slower, exacter.
S_DTYPE = os.environ.get("ATTN_S_DTYPE", "fp16")  # fp16 | bf16 | f32r | f32
NO_SPECIALIZE = os.environ.get("ATTN_NO_SPECIALIZE", "0") == "1"

LAST_RESULTS = None
_NC_CACHE: dict = {}


def _dtypes(sdt):
    """(qk_dt for Q/K/S-matmul, ldt for P/V)."""
    f32 = mybir.dt.float32
    qk = {"fp16": mybir.dt.float16, "bf16": mybir.dt.bfloat16,
          "f32r": mybir.dt.float32r, "f32": f32}[sdt]
    ldt = mybir.dt.float16 if sdt == "fp16" else mybir.dt.bfloat16
    return qk, ldt


def _body(tc, qT, kT, vsh, mbias, outT, den, kb_counts, sdt):
    nc = tc.nc
    f32 = mybir.dt.float32
    AF = mybir.ActivationFunctionType
    ADD = mybir.AluOpType.add
    qk_dt, ldt = _dtypes(sdt)

    with (
        tc.tile_pool(name="qk", bufs=3) as qk_pool,
        tc.tile_pool(name="v", bufs=3) as v_pool,
        tc.tile_pool(name="p", bufs=2) as p_pool,
        tc.tile_pool(name="acc", bufs=2) as a_pool,
        tc.tile_pool(name="eps", bufs=2) as e_pool,
        tc.tile_pool(name="const", bufs=1) as c_pool,
        tc.tile_pool(name="spsum", bufs=2, space="PSUM") as s_pool,
        tc.tile_pool(name="opsum", bufs=1, space="PSUM") as o_pool,
        tc.tile_pool(name="dpsum", bufs=1, space="PSUM") as d_pool,
    ):
        # Constants via DVE memset: the DVE queue is idle during the preamble
        # while sync/gpsimd are busy issuing the batch-0 DMAs.
        ones = c_pool.tile([128, 1], qk_dt, tag="ones")
        nc.vector.memset(ones[:], 1.0)
        wsrc = c_pool.tile([128, 256], qk_dt, tag="wsrc")
        nc.vector.memset(wsrc[:], 0.0)
        mb_sb = c_pool.tile([128, BPC * KB_MAX], f32, tag="mb")

        def load_batch(b):
            # one dma_start per tensor: descriptors of a single DMA already
            # spread across all 16 DMA engines, and each dma_start costs
            # ~620ns of issuing-engine time, so fewer instructions win.
            # Batch 0 is latency-critical: k-block-0 (32KB) and q-half-0 go
            # down FIRST on two different issue queues (scalar is idle until
            # the first exp, so it donates its queue) so the first S matmul
            # starts as early as possible.
            KB = kb_counts[b]
            KC = KB * 128
            q_sb = qk_pool.tile([128, Q], qk_dt, tag="q", name=f"q_sb{b}")
            k_sb = qk_pool.tile([128, KC], qk_dt, tag="k", name=f"k_sb{b}")
            v_sb = v_pool.tile([128, KC], ldt, tag="v", name=f"v_sb{b}")
            if b == 0:
                nc.scalar.dma_start(out=k_sb[:, 0:128], in_=kT[b][:, 0:128])
                nc.sync.dma_start(out=q_sb[:, 0:QH], in_=qT[b][:, 0:QH])
                if KC > 128:
                    nc.scalar.dma_start(out=k_sb[:, 128:KC],
                                        in_=kT[b][:, 128:KC])
                nc.sync.dma_start(out=q_sb[:, QH:Q], in_=qT[b][:, QH:Q])
                nc.gpsimd.dma_start(out=mb_sb[:], in_=mbias)
                nc.gpsimd.dma_start(out=v_sb[:], in_=vsh[b][:, 0:KC])
            else:
                nc.sync.dma_start(out=q_sb[:], in_=qT[b])
                nc.sync.dma_start(out=k_sb[:], in_=kT[b][:, 0:KC])
                nc.gpsimd.dma_start(out=v_sb[:], in_=vsh[b][:, 0:KC])
            return q_sb, k_sb, v_sb

        def s_exp_stage(b, q_sb, k_sb):
            KB = kb_counts[b]
            p_tiles = []
            den_srcs = []  # pair-sums (DVE) + a trailing odd P tile
            for kb in range(KB):
                s_ps = s_pool.tile([128, Q], f32, tag="s", name=f"s_ps{b}_{kb}")
                lhsT = k_sb[:, kb * 128 : (kb + 1) * 128]
                for qh in range(Q // QH):
                    nc.tensor.matmul(
                        s_ps[:, qh * QH : (qh + 1) * QH],
                        lhsT,
                        q_sb[:, qh * QH : (qh + 1) * QH],
                        start=True,
                        stop=True,
                    )
                p_t = p_pool.tile([128, Q], ldt, tag=f"p{kb}", name=f"p{b}_{kb}")
                col = b * KB_MAX + kb
                nc.scalar.activation(p_t[:], s_ps[:], AF.Exp,
                                     bias=mb_sb[:, col : col + 1], scale=SCALE)
                p_tiles.append(p_t)
                # pairwise denominator partial sums on DVE (787ns each); the
                # den matmul then PSUM-accumulates over the pair-sums, which
                # halves the DVE add count vs a full tree at +1024 PE rows
                # per pair
                if kb % 2 == 1:
                    j = kb // 2
                    ps = a_pool.tile([128, Q], ldt, tag=f"a{j}",
                                     name=f"psum{b}_{j}")
                    nc.vector.tensor_tensor(ps[:], p_tiles[kb - 1][:],
                                            p_t[:], ADD)
                    den_srcs.append(ps)
            if KB % 2 == 1:
                den_srcs.append(p_tiles[KB - 1])
            return p_tiles, den_srcs

        def den_pv_stage(b, p_tiles, v_sb, den_srcs):
            KB = kb_counts[b]
            last = b == BPC - 1

            def den_mms():
                # den[1, q] = ones.T @ sum P, PSUM-accumulated over pair-sums
                d_ps = d_pool.tile([1, Q], f32, tag="d", name=f"d_ps{b}")
                for j, src in enumerate(den_srcs):
                    for qh in range(Q // QH):
                        nc.tensor.matmul(
                            d_ps[:, qh * QH : (qh + 1) * QH],
                            ones[:, 0:1],
                            src[:, qh * QH : (qh + 1) * QH],
                            start=(j == 0),
                            stop=(j == len(den_srcs) - 1),
                        )
                den_sb = e_pool.tile([1, Q], f32, tag="densb",
                                     name=f"den_sb{b}")
                nc.vector.tensor_copy(den_sb[:], d_ps[:])
                nc.gpsimd.dma_start(out=den[b], in_=den_sb[:])

            # O^T[v, q] accumulated over k-blocks, V stationary (kb-outer);
            # the last batch goes qh-outer so qh0's copy+DMA overlaps qh1's
            # matmuls instead of serializing after the final matmul.
            o_ps = [o_pool.tile([128, QH], f32, tag=f"o{qh}", name=f"o_ps{b}_{qh}")
                    for qh in range(Q // QH)]
            o_all = e_pool.tile([128, Q], ldt, tag="oall", name=f"o_all{b}")

            def pv(kb, qh):
                nc.tensor.matmul(
                    o_ps[qh][:],
                    v_sb[:, kb * 128 : (kb + 1) * 128],
                    p_tiles[kb][:, qh * QH : (qh + 1) * QH],
                    start=(kb == 0),
                    stop=(kb == KB - 1),
                )

            def evac(qh, eng, dma_eng):
                # fp16 conversion halves the output DMA bytes; the host
                # divides by den in f32 anyway.
                if eng is nc.scalar:
                    eng.copy(o_all[:, qh * QH : (qh + 1) * QH], o_ps[qh][:])
                else:
                    eng.tensor_copy(
                        o_all[:, qh * QH : (qh + 1) * QH], o_ps[qh][:])
                dma_eng.dma_start(
                    out=outT[b][:, qh * QH : (qh + 1) * QH],
                    in_=o_all[:, qh * QH : (qh + 1) * QH])

            if last:
                # den between the two PV chains: the pair-sums are certainly
                # done by then (no PE stall) and its evac+DMA overlaps qh1
                for kb in range(KB):
                    pv(kb, 0)
                evac(0, nc.vector, nc.sync)
                den_mms()
                for kb in range(KB):
                    pv(kb, 1)
                # qh1's evac+DMA on ScalarE (done with exps by now) so the
                # tail copy and DMA issue don't queue behind anything
                evac(1, nc.scalar, nc.scalar)
            else:
                # PV first: it only needs P tiles (ScalarE-paced), while den
                # needs the last pair-sum — ordering den after PV means the
                # in-order PE queue never stalls on the DVE adds
                for kb in range(KB):
                    for qh in range(Q // QH):
                        pv(kb, qh)
                den_mms()
                for qh in range(Q // QH):
                    evac(qh, nc.vector, nc.sync)

        # PE p-state warmup: the PE ramps to 2.4GHz after ~3us of continuous
        # execution.  Small [1,256] matmuls (ones stationary, zero source)
        # into the den PSUM bank keep it busy from ~6.5us until batch-0 data
        # lands, with 1-row LDWEIGHTS and no PSUM pressure on the S banks.
        warm = d_pool.tile([1, Q], f32, tag="d", name="warm")
        for _ in range(N_WARM):
            nc.tensor.matmul(warm[:, 0:256], ones[:, 0:1], wsrc[:, 0:256],
                             start=True, stop=True)

        # Software pipeline: S+exp of batch b overlaps den/PV of batch b-1 on
        # the PE, so the ScalarE exp stream never gates the PE at batch
        # boundaries.
        prev = None
        for b in range(BPC):
            q_sb, k_sb, v_sb = load_batch(b)
            p_tiles, den_srcs = s_exp_stage(b, q_sb, k_sb)
            if prev is not None:
                den_pv_stage(*prev)
            prev = (b, p_tiles, v_sb, den_srcs)
        den_pv_stage(*prev)


def _build(kb_counts, sdt):
    key = (tuple(kb_counts), sdt)
    if key in _NC_CACHE:
        return _NC_CACHE[key]
    nc = bacc.Bacc("TRN2", target_bir_lowering=False, debug=False,
                   enable_asserts=False, enable_partition_id=False)
    f32 = mybir.dt.float32
    qk_dt, ldt = _dtypes(sdt)
    qT = nc.dram_tensor("qT", [BPC, D, Q], qk_dt, kind="ExternalInput").ap()
    kT = nc.dram_tensor("kT", [BPC, D, K], qk_dt, kind="ExternalInput").ap()
    vsh = nc.dram_tensor("vsh", [BPC, 128, KB_MAX * D], ldt,
                         kind="ExternalInput").ap()
    mbias = nc.dram_tensor("mbias", [128, BPC * KB_MAX], f32,
                           kind="ExternalInput").ap()
    outT = nc.dram_tensor("outT", [BPC, D, Q], ldt, kind="ExternalOutput").ap()
    den = nc.dram_tensor("den", [BPC, 1, Q], f32, kind="ExternalOutput").ap()
    with tile.TileContext(nc) as tc:
        _body(tc, qT, kT, vsh, mbias, outT, den, kb_counts, sdt)
    nc.compile()
    _NC_CACHE[key] = nc
    return nc


def _prep(queries, keys, values, valid_length):
    """Returns (in_maps, assign, kb_counts). assign[j, c] = original batch index
    handled by core c slot j."""
    vl = np.asarray(valid_length).astype(np.int64).reshape(B)
    if NO_SPECIALIZE:
        assign = np.arange(B).reshape(N_CORES, BPC).T
        kb_counts = tuple([KB_MAX] * BPC)
    else:
        order = np.argsort(-vl, kind="stable")
        assign = order.reshape(BPC, N_CORES)  # [slot, core]
        kb_counts = tuple(
            max(1, int(np.ceil(vl[assign[j]].max() / 128.0))) for j in range(BPC)
        )

    qk_np = {"fp16": np.float16, "bf16": ml_dtypes.bfloat16,
             "f32r": np.float32, "f32": np.float32}[S_DTYPE]
    ldt_np = np.float16 if S_DTYPE == "fp16" else ml_dtypes.bfloat16
    q = np.asarray(queries, dtype=np.float32)
    k = np.asarray(keys, dtype=np.float32)
    v = np.asarray(values, dtype=np.float32)
    karr = np.arange(K).reshape(KB_MAX, 128)  # [kb, p]

    in_maps = []
    for c in range(N_CORES):
        bidx = assign[:, c]
        qTc = np.ascontiguousarray(q[bidx].transpose(0, 2, 1)).astype(qk_np)
        kTc = np.ascontiguousarray(k[bidx].transpose(0, 2, 1)).astype(qk_np)
        vshc = np.ascontiguousarray(
            v[bidx].reshape(BPC, KB_MAX, 128, D).transpose(0, 2, 1, 3).reshape(
                BPC, 128, KB_MAX * D)
        ).astype(ldt_np)
        # mbias[p, b*KB_MAX+kb] = 0 where kb*128+p < vl else -1e9
        mb = np.where(karr[None] < vl[bidx][:, None, None], 0.0, -1e9)
        mbc = np.ascontiguousarray(
            mb.transpose(2, 0, 1).reshape(128, BPC * KB_MAX)).astype(np.float32)
        in_maps.append({"qT": qTc, "kT": kTc, "vsh": vshc, "mbias": mbc})
    return in_maps, assign, kb_counts


def kernel(queries, keys, values, valid_length):
    global LAST_RESULTS
    in_maps, assign, kb_counts = _prep(queries, keys, values, valid_length)
    nc = _build(kb_counts, S_DTYPE)
    res = run_bass_kernel_spmd(nc, in_maps, list(range(N_CORES)))
    LAST_RESULTS = res
    out = np.empty((B, Q, D), np.float32)
    for c in range(N_CORES):
        oT = np.asarray(res.results[c]["outT"]).astype(np.float32)  # [BPC,D,Q]
        den = np.asarray(res.results[c]["den"], dtype=np.float32)  # [BPC, 1, Q]
        o = (oT / den).transpose(0, 2, 1)
        for j in range(BPC):
            out[assign[j, c]] = o[j]
    return out


# revision 23
# speedup vs baseline: 1.1361x; 1.1361x over previous
"""Trainium2 Bass/Tile kernel: batched dot-product attention with length masking.

Problem: queries/keys/values [32, 1024, 128] f32, valid_length [32] int64.
  out = softmax(mask(Q K^T / sqrt(128))) @ V

Strategy:
  - Data-parallel: 32 batches sharded 4-per-core across 8 NeuronCores (SPMD,
    identical program, per-core input maps).
  - Host prep per batch (layout only, so every DMA moves 2-4KB contiguous
    chunks per partition):
      qT/kT = Q^T/K^T    [128=D, 1024] (contraction dim on partitions)
      vsh[p, kb, v] = (V * rowmask)[kb*128+p, v]  fp16, partition-major
  - Device per batch (default variant, exp-stream-bound at ~46us):
      S^T[k, q] = (K^T_kb).T @ Q^T       fp16 matmul, full PE rate
      P^T_kb    = exp(S^T/sqrt(D) + bias) ScalarE, PSUM->SBUF, fp16.  The
                  per-partition bias column zeroes masked k rows exactly
                  (exp(-1e9) == 0), so no mask matmul pass is needed.  The
                  21-tile exp stream (~1147ns each) is the critical path;
                  s-pool bufs=3 keeps it saturated.
      den[1,q]  = ones.T @ (DVE pairwise sums of P tiles), accumulated in
                  PSUM — ~1/8 the PE cost of the old mask-stationary pass
      O^T[v,q]  = sum_kb V_kb @ P^T_kb   (PE, V stationary)
    O^T (unnormalized) and den are DMAed out; the host does out = O^T.T/den.
    The last batch borrows freed s-pool PSUM banks for O/den so its tail
    never serializes on single-bank WAR evacs.
  - ~13.5us is fixed framework overhead (7.2 preamble + 6.3 drain epilogue);
    batch-0 gating bytes (k-block-0 + q halves) get their own DMA rings and
    6 wide warmup matmuls keep the PE p-state ramping until data lands.
  - Length specialization: batches sorted by valid_length desc, assigned
    round-robin so slot j is similar across cores; program compiled per
    kb_counts skips fully-masked k-blocks.

VARIANT knobs (module-level dict, also settable for sweeps):
  warm:  "wide9" 9x[128,512] warmup matmuls | "narrow" N x [1,256] | "none"
  den:   "matmul" per-kb mask-stationary pass | "pairsum" DVE pair adds +
         ones-stationary accumulated matmul (needs exp bias masking)
  load0: "classic" batch-0 k halves on gpsimd, q halves on sync
  tail:  "classic" kb-outer last batch | "qhouter" qh-outer last batch with
         scalar-queue output DMA
"""

import os

import numpy as np
import ml_dtypes

import concourse.tile as tile
from concourse import bacc, mybir
from concourse.bass_utils import run_bass_kernel_spmd

B, Q, K, D = 32, 1024, 1024, 128
N_CORES = 8
BPC = B // N_CORES  # batches per core
KB_MAX = K // 128
QH = 512
SCALE = float(1.0 / np.sqrt(D))

S_DTYPE = os.environ.get("ATTN_S_DTYPE", "fp16")  # fp16 | bf16 | f32r | f32
NO_SPECIALIZE = os.environ.get("ATTN_NO_SPECIALIZE", "0") == "1"

VARIANT = {
    "warm": "wide9",
    "warm_n": 6,
    "den": "pairsum3",
    "load0": "spread",
    "tail": "qhouter",
}

LAST_RESULTS = None
_NC_CACHE: dict = {}


def _dtypes(sdt):
    f32 = mybir.dt.float32
    qk = {"fp16": mybir.dt.float16, "bf16": mybir.dt.bfloat16,
          "f32r": mybir.dt.float32r, "f32": f32}[sdt]
    ldt = mybir.dt.float16 if sdt == "fp16" else mybir.dt.bfloat16
    return qk, ldt


def _body(tc, io, kb_counts, sdt, var):
    nc = tc.nc
    f32 = mybir.dt.float32
    AF = mybir.ActivationFunctionType
    ADD = mybir.AluOpType.add
    qk_dt, ldt = _dtypes(sdt)
    qT, kT, vsh = io["qT"], io["kT"], io["vsh"]
    outT, den = io["outT"], io["den"]
    ps3 = var["den"] == "pairsum3"
    pairsum = var["den"] == "pairsum" or ps3

    with (
        tc.tile_pool(name="qk", bufs=3) as qk_pool,
        tc.tile_pool(name="v", bufs=3) as v_pool,
        tc.tile_pool(name="p", bufs=2) as p_pool,
        tc.tile_pool(name="acc", bufs=2) as a_pool,
        tc.tile_pool(name="m", bufs=3) as m_pool,
        tc.tile_pool(name="eps", bufs=2) as e_pool,
        tc.tile_pool(name="const", bufs=1) as c_pool,
        tc.tile_pool(name="spsum", bufs=3 if ps3 else 2,
                     space="PSUM") as s_pool,
        tc.tile_pool(name="opsum", bufs=1, space="PSUM") as o_pool,
        tc.tile_pool(name="dpsum", bufs=1, space="PSUM") as d_pool,
    ):
        if pairsum:
            ones = c_pool.tile([128, 1], qk_dt, tag="ones")
            nc.vector.memset(ones[:], 1.0)
            mb_sb = c_pool.tile([128, BPC * KB_MAX], f32, tag="mb")
        if var["warm"] == "narrow":
            wsrc = c_pool.tile([128, 256], qk_dt, tag="wsrc")
            nc.vector.memset(wsrc[:], 0.0)
            if not pairsum:
                ones = c_pool.tile([128, 1], qk_dt, tag="ones")
                nc.vector.memset(ones[:], 1.0)

        def load_batch(b):
            KB = kb_counts[b]
            KC = KB * 128
            q_sb = qk_pool.tile([128, Q], qk_dt, tag="q", name=f"q_sb{b}")
            k_sb = qk_pool.tile([128, KC], qk_dt, tag="k", name=f"k_sb{b}")
            v_sb = v_pool.tile([128, KC], ldt, tag="v", name=f"v_sb{b}")
            m_sb = None
            if not pairsum:
                m_sb = m_pool.tile([128, KB], ldt, tag="mrow", name=f"m_sb{b}")
            if b == 0:
                h = KC // 2
                if var.get("wake"):
                    # tiny dummy reads to absorb the first-DMA wakeup
                    # latency on each issue queue before the real loads
                    wk = c_pool.tile([1, 8], qk_dt, tag="wake")
                    nc.sync.dma_start(out=wk[:, 0:4], in_=qT[0][0:1, 0:4])
                    nc.gpsimd.dma_start(out=wk[:, 4:8], in_=kT[0][0:1, 0:4])
                if var["load0"] == "spread":
                    # only k-block-0 (32KB) + q gate the first S pair: give
                    # each its own DMA ring so they land together ~2us
                    # before the bulk
                    nc.sync.dma_start(out=q_sb[:, 0:QH], in_=qT[b][:, 0:QH])
                    nc.scalar.dma_start(out=q_sb[:, QH:Q],
                                        in_=qT[b][:, QH:Q])
                    nc.gpsimd.dma_start(out=k_sb[:, 0:128],
                                        in_=kT[b][:, 0:128])
                    if KC > 128:
                        nc.gpsimd.dma_start(out=k_sb[:, 128:KC],
                                            in_=kT[b][:, 128:KC])
                    if pairsum:
                        nc.sync.dma_start(out=mb_sb[:], in_=io["mbias"])
                    else:
                        nc.sync.dma_start(out=m_sb[:],
                                          in_=io["mrow"][b][:, 0:KB])
                    nc.sync.dma_start(out=v_sb[:], in_=vsh[b][:, 0:KC])
                    return q_sb, k_sb, v_sb, m_sb
                # mbias is tiny (16KB) and gates the first exp: issue first
                if pairsum:
                    nc.gpsimd.dma_start(out=mb_sb[:], in_=io["mbias"])
                nc.gpsimd.dma_start(out=k_sb[:, 0:h], in_=kT[b][:, 0:h])
                nc.sync.dma_start(out=q_sb[:, 0:QH], in_=qT[b][:, 0:QH])
                nc.sync.dma_start(out=q_sb[:, QH:Q], in_=qT[b][:, QH:Q])
                nc.gpsimd.dma_start(out=k_sb[:, h:KC], in_=kT[b][:, h:KC])
                nc.sync.dma_start(out=v_sb[:], in_=vsh[b][:, 0:KC])
                if not pairsum:
                    nc.gpsimd.dma_start(out=m_sb[:], in_=io["mrow"][b][:, 0:KB])
            else:
                nc.sync.dma_start(out=q_sb[:], in_=qT[b])
                nc.sync.dma_start(out=k_sb[:], in_=kT[b][:, 0:KC])
                nc.gpsimd.dma_start(out=v_sb[:], in_=vsh[b][:, 0:KC])
                if not pairsum:
                    nc.gpsimd.dma_start(out=m_sb[:], in_=io["mrow"][b][:, 0:KB])
            return q_sb, k_sb, v_sb, m_sb

        def s_exp_stage(b, q_sb, k_sb):
            KB = kb_counts[b]
            p_tiles = []
            den_srcs = []
            for kb in range(KB):
                s_ps = s_pool.tile([128, Q], f32, tag="s", name=f"s_ps{b}_{kb}")
                lhsT = k_sb[:, kb * 128 : (kb + 1) * 128]
                for qh in range(Q // QH):
                    nc.tensor.matmul(
                        s_ps[:, qh * QH : (qh + 1) * QH],
                        lhsT,
                        q_sb[:, qh * QH : (qh + 1) * QH],
                        start=True,
                        stop=True,
                    )
                p_t = p_pool.tile([128, Q], ldt, tag=f"p{kb}", name=f"p{b}_{kb}")
                if pairsum:
                    col = b * KB_MAX + kb
                    nc.scalar.activation(p_t[:], s_ps[:], AF.Exp,
                                         bias=mb_sb[:, col : col + 1],
                                         scale=SCALE)
                    # pairwise den partial sums on DVE (787ns); the den
                    # matmul then PSUM-accumulates over them.  Each pair-sum
                    # lands ~0.8us after its second exp, i.e. before the
                    # NEXT exp finishes, so it never gates the den matmuls.
                    if kb % 2 == 1:
                        j = kb // 2
                        ps = a_pool.tile([128, Q], ldt, tag=f"a{j}",
                                         name=f"psum{b}_{j}")
                        nc.vector.tensor_tensor(ps[:], p_tiles[kb - 1][:],
                                                p_t[:], ADD)
                        den_srcs.append(ps)
                else:
                    nc.scalar.activation(p_t[:], s_ps[:], AF.Exp, scale=SCALE)
                p_tiles.append(p_t)
            if pairsum and KB % 2 == 1:
                den_srcs.append(p_tiles[KB - 1])
            return p_tiles, den_srcs

        def den_pv_stage(b, p_tiles, v_sb, m_sb, den_srcs):
            KB = kb_counts[b]
            last = b == BPC - 1
            qhouter = var["tail"] == "qhouter" and last

            def den_mms_pairsum():
                d_ps = d_pool.tile([1, Q], f32, tag="d", name=f"d_ps{b}")
                for j, src in enumerate(den_srcs):
                    for qh in range(Q // QH):
                        nc.tensor.matmul(
                            d_ps[:, qh * QH : (qh + 1) * QH],
                            ones[:, 0:1],
                            src[:, qh * QH : (qh + 1) * QH],
                            start=(j == 0),
                            stop=(j == len(den_srcs) - 1),
                        )
                den_sb = e_pool.tile([1, Q], f32, tag="densb",
                                     name=f"den_sb{b}")
                nc.vector.tensor_copy(den_sb[:], d_ps[:])
                nc.gpsimd.dma_start(out=den[b], in_=den_sb[:])

            def den_mms_matmul():
                d_ps = [d_pool.tile([1, QH], f32, tag=f"d{qh}",
                                    name=f"den_ps{b}_{qh}")
                        for qh in range(Q // QH)]
                for kb in range(KB):
                    for qh in range(Q // QH):
                        nc.tensor.matmul(
                            d_ps[qh][:],
                            m_sb[:, kb : kb + 1],
                            p_tiles[kb][:, qh * QH : (qh + 1) * QH],
                            start=(kb == 0),
                            stop=(kb == KB - 1),
                        )
                den_sb = e_pool.tile([1, Q], f32, tag="densb",
                                     name=f"den_sb{b}")
                for qh in range(Q // QH):
                    eng = nc.scalar if last else nc.vector
                    if eng is nc.scalar:
                        eng.copy(den_sb[:, qh * QH : (qh + 1) * QH],
                                 d_ps[qh][:])
                    else:
                        eng.tensor_copy(den_sb[:, qh * QH : (qh + 1) * QH],
                                        d_ps[qh][:])
                nc.gpsimd.dma_start(out=den[b], in_=den_sb[:])

            o_all = e_pool.tile([128, Q], ldt, tag="oall", name=f"o_all{b}")

            if ps3:
                den_sb = e_pool.tile([1, Q], f32, tag="densb",
                                     name=f"den_sb{b}")
                if last:
                    # after the last exp every s-pool bank is free: borrow
                    # one s tile's two banks as the two O accumulators and
                    # another for den, so PV kb-outer interleaves both qh
                    # chains right behind the exp stream and nothing
                    # serializes on evacs
                    o_l = s_pool.tile([128, Q], f32, tag="s", name="o_last")
                    d_l = s_pool.tile([128, Q], f32, tag="s", name="d_last")
                    for kb in range(KB):
                        for qh in range(Q // QH):
                            nc.tensor.matmul(
                                o_l[:, qh * QH : (qh + 1) * QH],
                                v_sb[:, kb * 128 : (kb + 1) * 128],
                                p_tiles[kb][:, qh * QH : (qh + 1) * QH],
                                start=(kb == 0),
                                stop=(kb == KB - 1),
                            )
                    for j, src in enumerate(den_srcs):
                        for qh in range(Q // QH):
                            nc.tensor.matmul(
                                d_l[0:1, qh * QH : (qh + 1) * QH],
                                ones[:, 0:1],
                                src[:, qh * QH : (qh + 1) * QH],
                                start=(j == 0),
                                stop=(j == len(den_srcs) - 1),
                            )
                    nc.vector.tensor_copy(o_all[:, 0:QH], o_l[:, 0:QH])
                    nc.sync.dma_start(out=outT[b][:, 0:QH],
                                      in_=o_all[:, 0:QH])
                    nc.scalar.copy(o_all[:, QH:Q], o_l[:, QH:Q])
                    nc.scalar.dma_start(out=outT[b][:, QH:Q],
                                        in_=o_all[:, QH:Q])
                    # den evac split across engines so neither copy
                    # serializes the final den DMA
                    nc.vector.tensor_copy(den_sb[:, 0:QH], d_l[0:1, 0:QH])
                    nc.scalar.copy(den_sb[:, QH:Q], d_l[0:1, QH:Q])
                    nc.gpsimd.dma_start(out=den[b], in_=den_sb[:])
                    return
                # single o PSUM bank, qh-outer; den per qh into a single
                # reused [1,512] bank, evac'd between qh's (ACT-paced slack
                # absorbs the WAR serialization mid-stream)
                for qh in range(Q // QH):
                    o_ps1 = o_pool.tile([128, QH], f32, tag="o",
                                        name=f"o_ps{b}_{qh}")
                    for kb in range(KB):
                        nc.tensor.matmul(
                            o_ps1[:],
                            v_sb[:, kb * 128 : (kb + 1) * 128],
                            p_tiles[kb][:, qh * QH : (qh + 1) * QH],
                            start=(kb == 0),
                            stop=(kb == KB - 1),
                        )
                    nc.vector.tensor_copy(
                        o_all[:, qh * QH : (qh + 1) * QH], o_ps1[:])
                    nc.sync.dma_start(
                        out=outT[b][:, qh * QH : (qh + 1) * QH],
                        in_=o_all[:, qh * QH : (qh + 1) * QH])
                    d_ps = d_pool.tile([1, QH], f32, tag="d0",
                                       name=f"d_ps{b}_{qh}")
                    for j, src in enumerate(den_srcs):
                        nc.tensor.matmul(
                            d_ps[:],
                            ones[:, 0:1],
                            src[:, qh * QH : (qh + 1) * QH],
                            start=(j == 0),
                            stop=(j == len(den_srcs) - 1),
                        )
                    nc.vector.tensor_copy(
                        den_sb[:, qh * QH : (qh + 1) * QH], d_ps[:])
                nc.gpsimd.dma_start(out=den[b], in_=den_sb[:])
                return

            o_ps = [o_pool.tile([128, QH], f32, tag=f"o{qh}",
                                name=f"o_ps{b}_{qh}")
                    for qh in range(Q // QH)]

            def pv(kb, qh):
                nc.tensor.matmul(
                    o_ps[qh][:],
                    v_sb[:, kb * 128 : (kb + 1) * 128],
                    p_tiles[kb][:, qh * QH : (qh + 1) * QH],
                    start=(kb == 0),
                    stop=(kb == KB - 1),
                )

            def evac(qh, eng, dma_eng):
                if eng is nc.scalar:
                    eng.copy(o_all[:, qh * QH : (qh + 1) * QH], o_ps[qh][:])
                else:
                    eng.tensor_copy(
                        o_all[:, qh * QH : (qh + 1) * QH], o_ps[qh][:])
                dma_eng.dma_start(
                    out=outT[b][:, qh * QH : (qh + 1) * QH],
                    in_=o_all[:, qh * QH : (qh + 1) * QH])

            if not pairsum:
                # mask-stationary den pass runs before PV (all P ready;
                # stationary loads once per kb)
                den_mms_matmul()
                if qhouter:
                    for qh in range(Q // QH):
                        for kb in range(KB):
                            pv(kb, qh)
                        evac(qh, nc.vector if qh == 0 else nc.scalar,
                             nc.sync if qh == 0 else nc.scalar)
                else:
                    for kb in range(KB):
                        for qh in range(Q // QH):
                            pv(kb, qh)
                    for qh in range(Q // QH):
                        evac(qh, nc.scalar if (last and qh == 1) else nc.vector,
                             nc.sync)
            else:
                if qhouter:
                    for kb in range(KB):
                        pv(kb, 0)
                    evac(0, nc.vector, nc.sync)
                    den_mms_pairsum()
                    for kb in range(KB):
                        pv(kb, 1)
                    evac(1, nc.scalar, nc.scalar)
                else:
                    for kb in range(KB):
                        for qh in range(Q // QH):
                            pv(kb, qh)
                    den_mms_pairsum()
                    for qh in range(Q // QH):
                        evac(qh, nc.scalar if (last and qh == 1) else nc.vector,
                             nc.sync)

        # PE p-state warmup
        if var["warm"] == "wide9":
            warm_w = e_pool.tile([128, QH], qk_dt, tag="warmw", bufs=1)
            nc.gpsimd.memset(warm_w[:], 0.0)
            for w in range(var["warm_n"]):
                warm_ps = s_pool.tile([128, QH], f32, tag="s", name=f"warm{w}")
                nc.tensor.matmul(warm_ps[:], warm_w[:, 0:128], warm_w[:],
                                 start=True, stop=True)
        elif var["warm"] == "narrow":
            wtag = "d" if pairsum else "d0"
            wshape = [1, Q] if pairsum else [1, QH]
            warm = d_pool.tile(wshape, f32, tag=wtag, name="warm")
            for _ in range(var["warm_n"]):
                nc.tensor.matmul(warm[:, 0:256], ones[:, 0:1], wsrc[:, 0:256],
                                 start=True, stop=True)

        prev = None
        for b in range(BPC):
            q_sb, k_sb, v_sb, m_sb = load_batch(b)
            p_tiles, den_srcs = s_exp_stage(b, q_sb, k_sb)
            if prev is not None:
                den_pv_stage(*prev)
            prev = (b, p_tiles, v_sb, m_sb, den_srcs)
        den_pv_stage(*prev)


def _build(kb_counts, sdt, var):
    key = (tuple(kb_counts), sdt, tuple(sorted(var.items())))
    if key in _NC_CACHE:
        return _NC_CACHE[key]
    nc = bacc.Bacc("TRN2", target_bir_lowering=False, debug=False,
                   enable_asserts=False, enable_partition_id=False)
    f32 = mybir.dt.float32
    qk_dt, ldt = _dtypes(sdt)
    io = {
        "qT": nc.dram_tensor("qT", [BPC, D, Q], qk_dt,
                             kind="ExternalInput").ap(),
        "kT": nc.dram_tensor("kT", [BPC, D, K], qk_dt,
                             kind="ExternalInput").ap(),
        "vsh": nc.dram_tensor("vsh", [BPC, 128, KB_MAX * D], ldt,
                              kind="ExternalInput").ap(),
        "outT": nc.dram_tensor("outT", [BPC, D, Q], ldt,
                               kind="ExternalOutput").ap(),
        "den": nc.dram_tensor("den", [BPC, 1, Q], f32,
                              kind="ExternalOutput").ap(),
    }
    if var["den"] in ("pairsum", "pairsum3"):
        io["mbias"] = nc.dram_tensor("mbias", [128, BPC * KB_MAX], f32,
                                     kind="ExternalInput").ap()
    else:
        io["mrow"] = nc.dram_tensor("mrow", [BPC, 128, KB_MAX], ldt,
                                    kind="ExternalInput").ap()
    with tile.TileContext(nc) as tc:
        _body(tc, io, kb_counts, sdt, var)
    nc.compile()
    _NC_CACHE[key] = nc
    return nc


def _prep(queries, keys, values, valid_length, var):
    vl = np.asarray(valid_length).astype(np.int64).reshape(B)
    if NO_SPECIALIZE:
        assign = np.arange(B).reshape(N_CORES, BPC).T
        kb_counts = tuple([KB_MAX] * BPC)
    else:
        order = np.argsort(-vl, kind="stable")
        assign = order.reshape(BPC, N_CORES)  # [slot, core]
        kb_counts = tuple(
            max(1, int(np.ceil(vl[assign[j]].max() / 128.0)))
            for j in range(BPC)
        )

    qk_np = {"fp16": np.float16, "bf16": ml_dtypes.bfloat16,
             "f32r": np.float32, "f32": np.float32}[S_DTYPE]
    ldt_np = np.float16 if S_DTYPE == "fp16" else ml_dtypes.bfloat16
    q = np.asarray(queries, dtype=np.float32)
    k = np.asarray(keys, dtype=np.float32)
    v = np.asarray(values, dtype=np.float32)
    karr = np.arange(K).reshape(KB_MAX, 128)  # [kb, p]
    pairsum = var["den"] in ("pairsum", "pairsum3")

    in_maps = []
    for c in range(N_CORES):
        bidx = assign[:, c]
        qTc = np.ascontiguousarray(q[bidx].transpose(0, 2, 1)).astype(qk_np)
        kTc = np.ascontiguousarray(k[bidx].transpose(0, 2, 1)).astype(qk_np)
        mask = (np.arange(K)[None, :] < vl[bidx][:, None]).astype(np.float32)
        vm = v[bidx] * mask[:, :, None]  # [BPC, K, D]
        vshc = np.ascontiguousarray(
            vm.reshape(BPC, KB_MAX, 128, D).transpose(0, 2, 1, 3).reshape(
                BPC, 128, KB_MAX * D)
        ).astype(ldt_np)
        m = {"qT": qTc, "kT": kTc, "vsh": vshc}
        if pairsum:
            mb = np.where(karr[None] < vl[bidx][:, None, None], 0.0, -1e9)
            m["mbias"] = np.ascontiguousarray(
                mb.transpose(2, 0, 1).reshape(128, BPC * KB_MAX)
            ).astype(np.float32)
        else:
            m["mrow"] = np.ascontiguousarray(
                mask.reshape(BPC, KB_MAX, 128).transpose(0, 2, 1)
            ).astype(ldt_np)
        in_maps.append(m)
    return in_maps, assign, kb_counts


def kernel(queries, keys, values, valid_length):
    global LAST_RESULTS
    var = dict(VARIANT)
    in_maps, assign, kb_counts = _prep(queries, keys, values, valid_length,
                                       var)
    nc = _build(kb_counts, S_DTYPE, var)
    res = run_bass_kernel_spmd(nc, in_maps, list(range(N_CORES)))
    LAST_RESULTS = res
    out = np.empty((B, Q, D), np.float32)
    for c in range(N_CORES):
        oT = np.asarray(res.results[c]["outT"]).astype(np.float32)
        den = np.asarray(res.results[c]["den"], dtype=np.float32)
        o = (oT / den).transpose(0, 2, 1)
        for j in range(BPC):
            out[assign[j, c]] = o[j]
    return out


# revision 27
# speedup vs baseline: 1.1853x; 1.0432x over previous
"""Trainium2 Bass/Tile kernel: batched dot-product attention with length masking.

Problem: queries/keys/values [32, 1024, 128] f32, valid_length [32] int64.
  out = softmax(mask(Q K^T / sqrt(128))) @ V

Strategy:
  - Data-parallel: 32 batches sharded 4-per-core across 8 NeuronCores (SPMD,
    identical program, per-core input maps).
  - Host prep per batch (layout only, so every DMA moves 2-4KB contiguous
    chunks per partition):
      qT/kT = Q^T/K^T    [128=D, 1024] (contraction dim on partitions)
      vsh[p, kb, v] = (V * rowmask)[kb*128+p, v]  fp16, partition-major
  - Device per batch (default variant, exp-stream-bound at ~46us):
      S^T[k, q] = (K^T_kb).T @ Q^T       fp16 matmul, full PE rate
      P^T_kb    = exp(S^T/sqrt(D) + bias) ScalarE, PSUM->SBUF, fp16.  The
                  per-partition bias column zeroes masked k rows exactly
                  (exp(-1e9) == 0), so no mask matmul pass is needed.  The
                  21-tile exp stream (~1147ns each) is the critical path;
                  s-pool bufs=3 keeps it saturated.
      den[1,q]  = ones.T @ (DVE pairwise sums of P tiles), accumulated in
                  PSUM — ~1/8 the PE cost of the old mask-stationary pass
      O^T[v,q]  = sum_kb V_kb @ P^T_kb   (PE, V stationary)
    O^T (unnormalized) and den are DMAed out; the host does out = O^T.T/den.
    The last batch borrows freed s-pool PSUM banks for O/den so its tail
    never serializes on single-bank WAR evacs.
  - ~13.5us is fixed framework overhead (7.2 preamble + 6.3 drain epilogue);
    batch-0 gating bytes (k-block-0 + q halves) get their own DMA rings and
    6 wide warmup matmuls keep the PE p-state ramping until data lands.
  - Length specialization: batches sorted by valid_length desc, assigned
    round-robin so slot j is similar across cores; program compiled per
    kb_counts skips fully-masked k-blocks.

VARIANT knobs (module-level dict, also settable for sweeps):
  warm:  "wide9" 9x[128,512] warmup matmuls | "narrow" N x [1,256] | "none"
  den:   "matmul" per-kb mask-stationary pass | "pairsum" DVE pair adds +
         ones-stationary accumulated matmul (needs exp bias masking)
  load0: "classic" batch-0 k halves on gpsimd, q halves on sync
  tail:  "classic" kb-outer last batch | "qhouter" qh-outer last batch with
         scalar-queue output DMA
"""

import os

import numpy as np
import ml_dtypes

import concourse.tile as tile
from concourse import bacc, mybir
from concourse.bass_utils import run_bass_kernel_spmd

B, Q, K, D = 32, 1024, 1024, 128
N_CORES = 8
BPC = B // N_CORES  # batches per core
KB_MAX = K // 128
QH = 512
SCALE = float(1.0 / np.sqrt(D))

S_DTYPE = os.environ.get("ATTN_S_DTYPE", "fp16")  # fp16 | bf16 | f32r | f32
NO_SPECIALIZE = os.environ.get("ATTN_NO_SPECIALIZE", "0") == "1"

VARIANT = {
    "warm": "wide9",
    "warm_n": 6,
    "den": "pairsum3",
    "load0": "spread",
    "tail": "qhouter",
}

LAST_RESULTS = None
_NC_CACHE: dict = {}


def _dtypes(sdt):
    f32 = mybir.dt.float32
    qk = {"fp16": mybir.dt.float16, "bf16": mybir.dt.bfloat16,
          "f32r": mybir.dt.float32r, "f32": f32}[sdt]
    ldt = mybir.dt.float16 if sdt == "fp16" else mybir.dt.bfloat16
    return qk, ldt


def _body(tc, io, kb_counts, sdt, var):
    nc = tc.nc
    f32 = mybir.dt.float32
    AF = mybir.ActivationFunctionType
    ADD = mybir.AluOpType.add
    qk_dt, ldt = _dtypes(sdt)
    qT, kT, vsh = io["qT"], io["kT"], io["vsh"]
    outT, den = io["outT"], io["den"]
    ps3 = var["den"] == "pairsum3"
    pairsum = var["den"] == "pairsum" or ps3

    with (
        tc.tile_pool(name="qk", bufs=3) as qk_pool,
        tc.tile_pool(name="v", bufs=3) as v_pool,
        tc.tile_pool(name="p", bufs=2) as p_pool,
        tc.tile_pool(name="acc", bufs=2) as a_pool,
        tc.tile_pool(name="m", bufs=3) as m_pool,
        tc.tile_pool(name="eps", bufs=2) as e_pool,
        tc.tile_pool(name="const", bufs=1) as c_pool,
        tc.tile_pool(name="spsum", bufs=3 if ps3 else 2,
                     space="PSUM") as s_pool,
        tc.tile_pool(name="opsum", bufs=1, space="PSUM") as o_pool,
        tc.tile_pool(name="dpsum", bufs=1, space="PSUM") as d_pool,
    ):
        if pairsum:
            ones = c_pool.tile([128, 1], qk_dt, tag="ones")
            nc.vector.memset(ones[:], 1.0)
            mb_sb = c_pool.tile([128, BPC * KB_MAX], f32, tag="mb")
        if var["warm"] == "narrow":
            wsrc = c_pool.tile([128, 256], qk_dt, tag="wsrc")
            nc.vector.memset(wsrc[:], 0.0)
            if not pairsum:
                ones = c_pool.tile([128, 1], qk_dt, tag="ones")
                nc.vector.memset(ones[:], 1.0)

        def load_batch(b):
            KB = kb_counts[b]
            KC = KB * 128
            q_sb = qk_pool.tile([128, Q], qk_dt, tag="q", name=f"q_sb{b}")
            k_sb = qk_pool.tile([128, KC], qk_dt, tag="k", name=f"k_sb{b}")
            v_sb = v_pool.tile([128, KC], ldt, tag="v", name=f"v_sb{b}")
            m_sb = None
            if not pairsum:
                m_sb = m_pool.tile([128, KB], ldt, tag="mrow", name=f"m_sb{b}")
            if b == 0:
                h = KC // 2
                if var.get("wake"):
                    # tiny dummy reads to absorb the first-DMA wakeup
                    # latency on each issue queue before the real loads
                    wk = c_pool.tile([1, 8], qk_dt, tag="wake")
                    nc.sync.dma_start(out=wk[:, 0:4], in_=qT[0][0:1, 0:4])
                    nc.gpsimd.dma_start(out=wk[:, 4:8], in_=kT[0][0:1, 0:4])
                if var["load0"] == "spread":
                    # only k-block-0 (32KB) + q gate the first S pair: give
                    # each its own DMA ring so they land together ~2us
                    # before the bulk.  k-rest follows qh1 on the otherwise
                    # quiet scalar ring so the exp stream (which consumes a
                    # k-block every ~1.15us from ~11.5us) never outruns it —
                    # on the gpsimd ring it contends with the b1+ loads.
                    nc.sync.dma_start(out=q_sb[:, 0:QH], in_=qT[b][:, 0:QH])
                    nc.scalar.dma_start(out=q_sb[:, QH:Q],
                                        in_=qT[b][:, QH:Q])
                    nc.gpsimd.dma_start(out=k_sb[:, 0:128],
                                        in_=kT[b][:, 0:128])
                    if KC > 128:
                        nc.scalar.dma_start(out=k_sb[:, 128:KC],
                                            in_=kT[b][:, 128:KC])
                    if pairsum:
                        nc.sync.dma_start(out=mb_sb[:], in_=io["mbias"])
                    else:
                        nc.sync.dma_start(out=m_sb[:],
                                          in_=io["mrow"][b][:, 0:KB])
                    nc.gpsimd.dma_start(out=v_sb[:], in_=vsh[b][:, 0:KC])
                    return q_sb, k_sb, v_sb, m_sb
                # mbias is tiny (16KB) and gates the first exp: issue first
                if pairsum:
                    nc.gpsimd.dma_start(out=mb_sb[:], in_=io["mbias"])
                nc.gpsimd.dma_start(out=k_sb[:, 0:h], in_=kT[b][:, 0:h])
                nc.sync.dma_start(out=q_sb[:, 0:QH], in_=qT[b][:, 0:QH])
                nc.sync.dma_start(out=q_sb[:, QH:Q], in_=qT[b][:, QH:Q])
                nc.gpsimd.dma_start(out=k_sb[:, h:KC], in_=kT[b][:, h:KC])
                nc.sync.dma_start(out=v_sb[:], in_=vsh[b][:, 0:KC])
                if not pairsum:
                    nc.gpsimd.dma_start(out=m_sb[:], in_=io["mrow"][b][:, 0:KB])
            else:
                nc.sync.dma_start(out=q_sb[:], in_=qT[b])
                nc.sync.dma_start(out=k_sb[:], in_=kT[b][:, 0:KC])
                nc.gpsimd.dma_start(out=v_sb[:], in_=vsh[b][:, 0:KC])
                if not pairsum:
                    nc.gpsimd.dma_start(out=m_sb[:], in_=io["mrow"][b][:, 0:KB])
            return q_sb, k_sb, v_sb, m_sb

        def s_exp_stage(b, q_sb, k_sb):
            KB = kb_counts[b]
            p_tiles = []
            den_srcs = []
            for kb in range(KB):
                s_ps = s_pool.tile([128, Q], f32, tag="s", name=f"s_ps{b}_{kb}")
                lhsT = k_sb[:, kb * 128 : (kb + 1) * 128]
                for qh in range(Q // QH):
                    nc.tensor.matmul(
                        s_ps[:, qh * QH : (qh + 1) * QH],
                        lhsT,
                        q_sb[:, qh * QH : (qh + 1) * QH],
                        start=True,
                        stop=True,
                    )
                p_t = p_pool.tile([128, Q], ldt, tag=f"p{kb}", name=f"p{b}_{kb}")
                if pairsum:
                    col = b * KB_MAX + kb
                    nc.scalar.activation(p_t[:], s_ps[:], AF.Exp,
                                         bias=mb_sb[:, col : col + 1],
                                         scale=SCALE)
                    # pairwise den partial sums on DVE (787ns); the den
                    # matmul then PSUM-accumulates over them.  Each pair-sum
                    # lands ~0.8us after its second exp, i.e. before the
                    # NEXT exp finishes, so it never gates the den matmuls.
                    if kb % 2 == 1:
                        j = kb // 2
                        ps = a_pool.tile([128, Q], ldt, tag=f"a{j}",
                                         name=f"psum{b}_{j}")
                        nc.vector.tensor_tensor(ps[:], p_tiles[kb - 1][:],
                                                p_t[:], ADD)
                        den_srcs.append(ps)
                else:
                    nc.scalar.activation(p_t[:], s_ps[:], AF.Exp, scale=SCALE)
                p_tiles.append(p_t)
            if pairsum and KB % 2 == 1:
                den_srcs.append(p_tiles[KB - 1])
            return p_tiles, den_srcs

        def den_pv_stage(b, p_tiles, v_sb, m_sb, den_srcs):
            KB = kb_counts[b]
            last = b == BPC - 1
            qhouter = var["tail"] == "qhouter" and last

            def den_mms_pairsum():
                d_ps = d_pool.tile([1, Q], f32, tag="d", name=f"d_ps{b}")
                for j, src in enumerate(den_srcs):
                    for qh in range(Q // QH):
                        nc.tensor.matmul(
                            d_ps[:, qh * QH : (qh + 1) * QH],
                            ones[:, 0:1],
                            src[:, qh * QH : (qh + 1) * QH],
                            start=(j == 0),
                            stop=(j == len(den_srcs) - 1),
                        )
                den_sb = e_pool.tile([1, Q], f32, tag="densb",
                                     name=f"den_sb{b}")
                nc.vector.tensor_copy(den_sb[:], d_ps[:])
                nc.gpsimd.dma_start(out=den[b], in_=den_sb[:])

            def den_mms_matmul():
                d_ps = [d_pool.tile([1, QH], f32, tag=f"d{qh}",
                                    name=f"den_ps{b}_{qh}")
                        for qh in range(Q // QH)]
                for kb in range(KB):
                    for qh in range(Q // QH):
                        nc.tensor.matmul(
                            d_ps[qh][:],
                            m_sb[:, kb : kb + 1],
                            p_tiles[kb][:, qh * QH : (qh + 1) * QH],
                            start=(kb == 0),
                            stop=(kb == KB - 1),
                        )
                den_sb = e_pool.tile([1, Q], f32, tag="densb",
                                     name=f"den_sb{b}")
                for qh in range(Q // QH):
                    eng = nc.scalar if last else nc.vector
                    if eng is nc.scalar:
                        eng.copy(den_sb[:, qh * QH : (qh + 1) * QH],
                                 d_ps[qh][:])
                    else:
                        eng.tensor_copy(den_sb[:, qh * QH : (qh + 1) * QH],
                                        d_ps[qh][:])
                nc.gpsimd.dma_start(out=den[b], in_=den_sb[:])

            o_all = e_pool.tile([128, Q], ldt, tag="oall", name=f"o_all{b}")

            if ps3:
                den_sb = e_pool.tile([1, Q], f32, tag="densb",
                                     name=f"den_sb{b}")
                if last:
                    # after the last exp every s-pool bank is free: borrow
                    # one s tile's two banks as the two O accumulators and
                    # another for den, so PV kb-outer interleaves both qh
                    # chains right behind the exp stream and nothing
                    # serializes on evacs
                    o_l = s_pool.tile([128, Q], f32, tag="s", name="o_last")
                    d_l = s_pool.tile([128, Q], f32, tag="s", name="d_last")

                    def den_l(js):
                        for j in js:
                            for qh in range(Q // QH):
                                nc.tensor.matmul(
                                    d_l[0:1, qh * QH : (qh + 1) * QH],
                                    ones[:, 0:1],
                                    den_srcs[j][:, qh * QH : (qh + 1) * QH],
                                    start=(j == 0),
                                    stop=(j == len(den_srcs) - 1),
                                )

                    # all-but-last den sources are ready before the final
                    # exp: emit them first so only 2 den matmuls trail it
                    den_l(range(len(den_srcs) - 1))
                    for kb in range(KB):
                        for qh in range(Q // QH):
                            nc.tensor.matmul(
                                o_l[:, qh * QH : (qh + 1) * QH],
                                v_sb[:, kb * 128 : (kb + 1) * 128],
                                p_tiles[kb][:, qh * QH : (qh + 1) * QH],
                                start=(kb == 0),
                                stop=(kb == KB - 1),
                            )
                    den_l([len(den_srcs) - 1])
                    nc.vector.tensor_copy(o_all[:, 0:QH], o_l[:, 0:QH])
                    nc.sync.dma_start(out=outT[b][:, 0:QH],
                                      in_=o_all[:, 0:QH])
                    nc.scalar.copy(o_all[:, QH:Q], o_l[:, QH:Q])
                    nc.scalar.dma_start(out=outT[b][:, QH:Q],
                                        in_=o_all[:, QH:Q])
                    # den evac split across engines; final den DMA on the
                    # idle sync HWDGE ring (gpsimd SWDGE adds ~1us that the
                    # drain epilogue would wait out)
                    nc.vector.tensor_copy(den_sb[:, 0:QH], d_l[0:1, 0:QH])
                    nc.scalar.copy(den_sb[:, QH:Q], d_l[0:1, QH:Q])
                    nc.sync.dma_start(out=den[b], in_=den_sb[:])
                    return
                # single o PSUM bank, qh-outer; den per qh into a single
                # reused [1,512] bank, evac'd between qh's (ACT-paced slack
                # absorbs the WAR serialization mid-stream)
                for qh in range(Q // QH):
                    o_ps1 = o_pool.tile([128, QH], f32, tag="o",
                                        name=f"o_ps{b}_{qh}")
                    for kb in range(KB):
                        nc.tensor.matmul(
                            o_ps1[:],
                            v_sb[:, kb * 128 : (kb + 1) * 128],
                            p_tiles[kb][:, qh * QH : (qh + 1) * QH],
                            start=(kb == 0),
                            stop=(kb == KB - 1),
                        )
                    nc.vector.tensor_copy(
                        o_all[:, qh * QH : (qh + 1) * QH], o_ps1[:])
                    nc.sync.dma_start(
                        out=outT[b][:, qh * QH : (qh + 1) * QH],
                        in_=o_all[:, qh * QH : (qh + 1) * QH])
                    d_ps = d_pool.tile([1, QH], f32, tag="d0",
                                       name=f"d_ps{b}_{qh}")
                    for j, src in enumerate(den_srcs):
                        nc.tensor.matmul(
                            d_ps[:],
                            ones[:, 0:1],
                            src[:, qh * QH : (qh + 1) * QH],
                            start=(j == 0),
                            stop=(j == len(den_srcs) - 1),
                        )
                    nc.vector.tensor_copy(
                        den_sb[:, qh * QH : (qh + 1) * QH], d_ps[:])
                nc.gpsimd.dma_start(out=den[b], in_=den_sb[:])
                return

            o_ps = [o_pool.tile([128, QH], f32, tag=f"o{qh}",
                                name=f"o_ps{b}_{qh}")
                    for qh in range(Q // QH)]

            def pv(kb, qh):
                nc.tensor.matmul(
                    o_ps[qh][:],
                    v_sb[:, kb * 128 : (kb + 1) * 128],
                    p_tiles[kb][:, qh * QH : (qh + 1) * QH],
                    start=(kb == 0),
                    stop=(kb == KB - 1),
                )

            def evac(qh, eng, dma_eng):
                if eng is nc.scalar:
                    eng.copy(o_all[:, qh * QH : (qh + 1) * QH], o_ps[qh][:])
                else:
                    eng.tensor_copy(
                        o_all[:, qh * QH : (qh + 1) * QH], o_ps[qh][:])
                dma_eng.dma_start(
                    out=outT[b][:, qh * QH : (qh + 1) * QH],
                    in_=o_all[:, qh * QH : (qh + 1) * QH])

            if not pairsum:
                # mask-stationary den pass runs before PV (all P ready;
                # stationary loads once per kb)
                den_mms_matmul()
                if qhouter:
                    for qh in range(Q // QH):
                        for kb in range(KB):
                            pv(kb, qh)
                        evac(qh, nc.vector if qh == 0 else nc.scalar,
                             nc.sync if qh == 0 else nc.scalar)
                else:
                    for kb in range(KB):
                        for qh in range(Q // QH):
                            pv(kb, qh)
                    for qh in range(Q // QH):
                        evac(qh, nc.scalar if (last and qh == 1) else nc.vector,
                             nc.sync)
            else:
                if qhouter:
                    for kb in range(KB):
                        pv(kb, 0)
                    evac(0, nc.vector, nc.sync)
                    den_mms_pairsum()
                    for kb in range(KB):
                        pv(kb, 1)
                    evac(1, nc.scalar, nc.scalar)
                else:
                    for kb in range(KB):
                        for qh in range(Q // QH):
                            pv(kb, qh)
                    den_mms_pairsum()
                    for qh in range(Q // QH):
                        evac(qh, nc.scalar if (last and qh == 1) else nc.vector,
                             nc.sync)

        # PE p-state warmup
        if var["warm"] == "wide9":
            # memset on DVE: it is idle in the preamble, while a gpsimd
            # memset would delay the k-block-0 DMA issue behind it
            warm_w = e_pool.tile([128, QH], qk_dt, tag="warmw", bufs=1)
            nc.vector.memset(warm_w[:], 0.0)
            for w in range(var["warm_n"]):
                # half-size final warm: finer drain granularity right when
                # batch-0 data lands
                cols = 256 if w == var["warm_n"] - 1 else QH
                warm_ps = s_pool.tile([128, QH], f32, tag="s", name=f"warm{w}")
                nc.tensor.matmul(warm_ps[:, 0:cols], warm_w[:, 0:128],
                                 warm_w[:, 0:cols], start=True, stop=True)
        elif var["warm"] == "narrow":
            wtag = "d" if pairsum else "d0"
            wshape = [1, Q] if pairsum else [1, QH]
            warm = d_pool.tile(wshape, f32, tag=wtag, name="warm")
            for _ in range(var["warm_n"]):
                nc.tensor.matmul(warm[:, 0:256], ones[:, 0:1], wsrc[:, 0:256],
                                 start=True, stop=True)

        prev = None
        for b in range(BPC):
            q_sb, k_sb, v_sb, m_sb = load_batch(b)
            p_tiles, den_srcs = s_exp_stage(b, q_sb, k_sb)
            if prev is not None:
                den_pv_stage(*prev)
            prev = (b, p_tiles, v_sb, m_sb, den_srcs)
        den_pv_stage(*prev)


def _build(kb_counts, sdt, var):
    key = (tuple(kb_counts), sdt, tuple(sorted(var.items())))
    if key in _NC_CACHE:
        return _NC_CACHE[key]
    nc = bacc.Bacc("TRN2", target_bir_lowering=False, debug=False,
                   enable_asserts=False, enable_partition_id=False)
    f32 = mybir.dt.float32
    qk_dt, ldt = _dtypes(sdt)
    io = {
        "qT": nc.dram_tensor("qT", [BPC, D, Q], qk_dt,
                             kind="ExternalInput").ap(),
        "kT": nc.dram_tensor("kT", [BPC, D, K], qk_dt,
                             kind="ExternalInput").ap(),
        "vsh": nc.dram_tensor("vsh", [BPC, 128, KB_MAX * D], ldt,
                              kind="ExternalInput").ap(),
        "outT": nc.dram_tensor("outT", [BPC, D, Q], ldt,
                               kind="ExternalOutput").ap(),
        "den": nc.dram_tensor("den", [BPC, 1, Q], f32,
                              kind="ExternalOutput").ap(),
    }
    if var["den"] in ("pairsum", "pairsum3"):
        io["mbias"] = nc.dram_tensor("mbias", [128, BPC * KB_MAX], f32,
                                     kind="ExternalInput").ap()
    else:
        io["mrow"] = nc.dram_tensor("mrow", [BPC, 128, KB_MAX], ldt,
                                    kind="ExternalInput").ap()
    with tile.TileContext(nc) as tc:
        _body(tc, io, kb_counts, sdt, var)
    nc.compile()
    _NC_CACHE[key] = nc
    return nc


def _prep(queries, keys, values, valid_length, var):
    vl = np.asarray(valid_length).astype(np.int64).reshape(B)
    if NO_SPECIALIZE:
        assign = np.arange(B).reshape(N_CORES, BPC).T
        kb_counts = tuple([KB_MAX] * BPC)
    else:
        order = np.argsort(-vl, kind="stable")
        assign = order.reshape(BPC, N_CORES)  # [slot, core]
        kb_counts = tuple(
            max(1, int(np.ceil(vl[assign[j]].max() / 128.0)))
            for j in range(BPC)
        )

    qk_np = {"fp16": np.float16, "bf16": ml_dtypes.bfloat16,
             "f32r": np.float32, "f32": np.float32}[S_DTYPE]
    ldt_np = np.float16 if S_DTYPE == "fp16" else ml_dtypes.bfloat16
    q = np.asarray(queries, dtype=np.float32)
    k = np.asarray(keys, dtype=np.float32)
    v = np.asarray(values, dtype=np.float32)
    karr = np.arange(K).reshape(KB_MAX, 128)  # [kb, p]
    pairsum = var["den"] in ("pairsum", "pairsum3")

    in_maps = []
    for c in range(N_CORES):
        bidx = assign[:, c]
        qTc = np.ascontiguousarray(q[bidx].transpose(0, 2, 1)).astype(qk_np)
        kTc = np.ascontiguousarray(k[bidx].transpose(0, 2, 1)).astype(qk_np)
        mask = (np.arange(K)[None, :] < vl[bidx][:, None]).astype(np.float32)
        vm = v[bidx] * mask[:, :, None]  # [BPC, K, D]
        vshc = np.ascontiguousarray(
            vm.reshape(BPC, KB_MAX, 128, D).transpose(0, 2, 1, 3).reshape(
                BPC, 128, KB_MAX * D)
        ).astype(ldt_np)
        m = {"qT": qTc, "kT": kTc, "vsh": vshc}
        if pairsum:
            mb = np.where(karr[None] < vl[bidx][:, None, None], 0.0, -1e9)
            m["mbias"] = np.ascontiguousarray(
                mb.transpose(2, 0, 1).reshape(128, BPC * KB_MAX)
            ).astype(np.float32)
        else:
            m["mrow"] = np.ascontiguousarray(
                mask.reshape(BPC, KB_MAX, 128).transpose(0, 2, 1)
            ).astype(ldt_np)
        in_maps.append(m)
    return in_maps, assign, kb_counts


def kernel(queries, keys, values, valid_length):
    global LAST_RESULTS
    var = dict(VARIANT)
    in_maps, assign, kb_counts = _prep(queries, keys, values, valid_length,
                                       var)
    nc = _build(kb_counts, S_DTYPE, var)
    res = run_bass_kernel_spmd(nc, in_maps, list(range(N_CORES)))
    LAST_RESULTS = res
    out = np.empty((B, Q, D), np.float32)
    for c in range(N_CORES):
        oT = np.asarray(res.results[c]["outT"]).astype(np.float32)
        den = np.asarray(res.results[c]["den"], dtype=np.float32)
        o = (oT / den).transpose(0, 2, 1)
        for j in range(BPC):
            out[assign[j, c]] = o[j]
    return out
